# revision 7
# baseline (speedup 1.0000x reference)
"""Trainium2 Bass kernel for GraphTransformerModel (2x TransformerConv, 4 heads).

Strategy: partition destination nodes across 8 cores (contiguous shards of
6250). Host sorts edges by dst into per-core, per-128-dst-block groups padded
to a uniform tile count T. Device gathers K|V rows (interleaved table) and Q
rows per edge via indirect DMA, computes exp(q.k/8) per head, and performs the
segment-softmax numerator/denominator with a one-hot(slot) matmul accumulated
in PSUM per dst block. The inter-conv dependency (conv2 K/V need all nodes'
post-conv1 features) is resolved with an AllGather collective.
"""
import sys
sys.path.insert(0, '/opt/trn_rl_repo')
import numpy as np

import concourse.bass as bass
import concourse.mybir as mybir
from concourse import bacc
from concourse.bass_utils import run_bass_kernel_spmd
from concourse.masks import make_identity
from concourse.tile import TileContext

P = 128
N, E, IN_C, HID, HEADS, OUT_C = 50000, 800000, 64, 64, 4, 32
C = HEADS * 64  # 256
NCORE = 8
NSHARD = N // NCORE          # 6250
NBLK = (NSHARD + P - 1) // P  # 49
LN_EPS = 1e-5
F32 = mybir.dt.float32
I32 = mybir.dt.int32

_CACHE = {}


def _ln_relu(nc, pool, out_sb, in_ap, nt, d, eps_c, g_bc, b_bc, do_g, do_b):
    """out_sb[:nt,:d] = relu(LN(in_ap[:nt,:d]) * g + b). in_ap may be PSUM."""
    st = pool.tile([P, 6], F32)
    mv = pool.tile([P, 2], F32)
    nc.vector.bn_stats(st[:nt], in_ap)
    nc.vector.bn_aggr(mv[:nt], st[:nt])
    rs = pool.tile([P, 1], F32)
    nc.scalar.activation(out=rs[:nt], in_=mv[:nt, 1:2],
                         func=mybir.ActivationFunctionType.Sqrt,
                         bias=eps_c[:nt], scale=1.0)
    nc.vector.reciprocal(rs[:nt], rs[:nt])
    mrs = pool.tile([P, 1], F32)
    nc.vector.tensor_tensor(out=mrs[:nt], in0=mv[:nt, 0:1], in1=rs[:nt],
                            op=mybir.AluOpType.mult)
    nc.vector.tensor_scalar(out=out_sb[:nt, :d], in0=in_ap,
                            scalar1=rs[:nt, 0:1], scalar2=mrs[:nt, 0:1],
                            op0=mybir.AluOpType.mult, op1=mybir.AluOpType.subtract)
    if do_g:
        nc.vector.tensor_tensor(out=out_sb[:nt, :d], in0=out_sb[:nt, :d],
                                in1=g_bc[:nt], op=mybir.AluOpType.mult)
    if do_b:
        nc.vector.tensor_tensor(out=out_sb[:nt, :d], in0=out_sb[:nt, :d],
                                in1=b_bc[:nt], op=mybir.AluOpType.add)
    nc.vector.tensor_scalar(out=out_sb[:nt, :d], in0=out_sb[:nt, :d],
                            scalar1=0.0, scalar2=None, op0=mybir.AluOpType.max)


def _build(T, ln_triv):
    nc = bacc.Bacc()
    dp = nc.declare_dram_parameter
    NT_E = NBLK * T  # edge tiles per core

    # ---- external inputs ----
    xT1 = dp("xT1", [IN_C + 1, N], F32, isOutput=False)   # x^T with ones row
    xTown = dp("xTown", [IN_C + 1, NSHARD], F32, isOutput=False)  # own-shard x^T
    Winp = dp("Winp", [IN_C + 1, HID], F32, isOutput=False)  # [W;b]
    Wkv1 = dp("Wkv1", [HID, 2 * C], F32, isOutput=False)
    bkv1 = dp("bkv1", [1, 2 * C], F32, isOutput=False)
    Wq1 = dp("Wq1", [HID, C], F32, isOutput=False)
    bq1 = dp("bq1", [1, C], F32, isOutput=False)
    Wsr1 = dp("Wsr1", [HID, 2 * C], F32, isOutput=False)
    bsr1 = dp("bsr1", [1, 2 * C], F32, isOutput=False)
    Wkv2 = dp("Wkv2", [P, 4 * C], F32, isOutput=False)  # [256,512] split
    bkv2 = dp("bkv2", [1, 2 * C], F32, isOutput=False)
    Wq2 = dp("Wq2", [P, 2 * C], F32, isOutput=False)
    bq2 = dp("bq2", [1, C], F32, isOutput=False)
    Wsr2 = dp("Wsr2", [P, 4 * C], F32, isOutput=False)
    bsr2 = dp("bsr2", [1, 2 * C], F32, isOutput=False)
    Wo1 = dp("Wo1", [P, 2 * HID], F32, isOutput=False)
    bo1 = dp("bo1", [1, HID], F32, isOutput=False)
    Wo2 = dp("Wo2", [HID, OUT_C], F32, isOutput=False)
    bo2 = dp("bo2", [1, OUT_C], F32, isOutput=False)
    lng = dp("lng", [4, C], F32, isOutput=False)   # rows: ln_in(64), ln1, ln2, ln_out(64) gammas (padded)
    lnb = dp("lnb", [4, C], F32, isOutput=False)
    srci = dp("srci", [P, NT_E], I32, isOutput=False)
    dsti = dp("dsti", [P, NT_E], I32, isOutput=False)
    slotf = dp("slotf", [P, NT_E], F32, isOutput=False)
    iota_d = dp("iota_d", [1, P], F32, isOutput=False)
    out_sh = dp("out_sh", [NBLK * P, OUT_C], F32, isOutput=True)

    # ---- internal DRAM ----
    kv1_tab = nc.dram_tensor("kv1_tab", [N, 2 * C], F32)
    q1_tab = nc.dram_tensor("q1_tab", [NBLK * P, C], F32)
    sr1_tab = nc.dram_tensor("sr1_tab", [NBLK * P, 2 * C], F32)
    h2_own = nc.dram_tensor("h2_own", [NSHARD, C], F32)
    h2_full = nc.dram_tensor("h2_full", [N, C], F32, addr_space="Shared")
    kv2_tab = nc.dram_tensor("kv2_tab", [N, 2 * C], F32)
    q2_tab = nc.dram_tensor("q2_tab", [NBLK * P, C], F32)
    sr2_tab = nc.dram_tensor("sr2_tab", [NBLK * P, 2 * C], F32)

    NTILE_N = (N + P - 1) // P  # 391
    core_id = None  # per-core data comes via in_maps; program is uniform

    with TileContext(nc) as tc:
        with tc.tile_pool(name="const", bufs=1) as cst, \
             tc.tile_pool(name="wpool", bufs=1) as wp, \
             tc.tile_pool(name="io", bufs=4) as io, \
             tc.tile_pool(name="ln", bufs=4) as lnp, \
             tc.tile_pool(name="edge", bufs=4) as ep, \
             tc.tile_pool(name="fin", bufs=3) as fp, \
             tc.tile_pool(name="ps_acc", bufs=2, space="PSUM") as ps_acc, \
             tc.tile_pool(name="ps_mm", bufs=3, space="PSUM") as ps_mm, \
             tc.tile_pool(name="ps_tr", bufs=3, space="PSUM") as ps_tr:

            # ---------- constants ----------
            ident = cst.tile([P, P], F32)
            make_identity(nc, ident[:])
            eps_c = cst.tile([P, 1], F32)
            nc.vector.memset(eps_c[:], LN_EPS)
            iota_bc = cst.tile([P, P], F32)
            nc.sync.dma_start(out=iota_bc[:], in_=iota_d[:].to_broadcast([P, P]))

            _uid = [0]

            def bcast(drow, w):
                _uid[0] += 1
                t = cst.tile([P, w], F32, tag=f"bc{_uid[0]}")
                nc.sync.dma_start(out=t[:], in_=drow[:1, :w].to_broadcast([P, w]))
                return t
            bkv1_bc = bcast(bkv1, 2 * C)
            bq1_bc = bcast(bq1, C)
            bsr1_bc = bcast(bsr1, 2 * C)
            bkv2_bc = bcast(bkv2, 2 * C)
            bq2_bc = bcast(bq2, C)
            bsr2_bc = bcast(bsr2, 2 * C)
            bo1_bc = bcast(bo1, HID)
            bo2_bc = bcast(bo2, OUT_C)
            g_bcs, b_bcs = [], []
            for r, w in enumerate((HID, C, C, HID)):
                if ln_triv[r]:
                    g_bcs.append(None); b_bcs.append(None)
                else:
                    tg = cst.tile([P, w], F32, tag=f"lng{r}")
                    nc.sync.dma_start(out=tg[:], in_=lng[r:r + 1, :w].to_broadcast([P, w]))
                    tb = cst.tile([P, w], F32, tag=f"lnb{r}")
                    nc.sync.dma_start(out=tb[:], in_=lnb[r:r + 1, :w].to_broadcast([P, w]))
                    g_bcs.append(tg); b_bcs.append(tb)

            # weights to SBUF
            def wtile(d, shape):
                _uid[0] += 1
                t = wp.tile(shape, F32, tag=f"w{_uid[0]}")
                nc.sync.dma_start(out=t[:], in_=d[:])
                return t
            Winp_sb = wtile(Winp, [IN_C + 1, HID])
            Wkv1_sb = wtile(Wkv1, [HID, 2 * C])
            Wq1_sb = wtile(Wq1, [HID, C])
            Wsr1_sb = wtile(Wsr1, [HID, 2 * C])
            Wkv2_sb = wtile(Wkv2, [P, 4 * C])
            Wq2_sb = wtile(Wq2, [P, 2 * C])
            Wsr2_sb = wtile(Wsr2, [P, 4 * C])
            Wo1_sb = wtile(Wo1, [P, 2 * HID])
            Wo2_sb = wtile(Wo2, [HID, OUT_C])

            # zero-fill pad rows of local tables (rows NSHARD..NBLK*P)
            npad = NBLK * P - NSHARD
            if npad > 0:
                zt = cst.tile([P, 2 * C], F32, tag="zpad")
                nc.vector.memset(zt[:], 0.0)
                nc.sync.dma_start(out=q1_tab[NSHARD:, :], in_=zt[:npad, :C])
                nc.sync.dma_start(out=sr1_tab[NSHARD:, :], in_=zt[:npad, :])
                nc.sync.dma_start(out=q2_tab[NSHARD:, :], in_=zt[:npad, :C])
                nc.sync.dma_start(out=sr2_tab[NSHARD:, :], in_=zt[:npad, :])
                nc.sync.dma_start(out=out_sh[NSHARD:, :], in_=zt[:npad, :OUT_C])

            # edge index arrays resident in SBUF
            src_sb = cst.tile([P, NT_E], I32)
            nc.sync.dma_start(out=src_sb[:], in_=srci[:])
            dst_sb = cst.tile([P, NT_E], I32)
            nc.sync.dma_start(out=dst_sb[:], in_=dsti[:])
            slot_sb = cst.tile([P, NT_E], F32)
            nc.sync.dma_start(out=slot_sb[:], in_=slotf[:])

            def h1_tile(g0, nt):
                """compute h1 rows [g0, g0+nt) -> (h1_sb, h1T_sb)"""
                xt = io.tile([IN_C + 1, P], F32, tag="xt")
                nc.sync.dma_start(out=xt[:, :nt], in_=xT1[:, g0:g0 + nt])
                ph = ps_mm.tile([P, HID], F32, space="PSUM", tag="mm")
                nc.tensor.matmul(out=ph[:nt], lhsT=xt[:, :nt], rhs=Winp_sb[:],
                                 start=True, stop=True)
                h1 = io.tile([P, HID], F32, tag="h1")
                _ln_relu(nc, lnp, h1, ph[:nt, :HID], nt, HID, eps_c,
                         g_bcs[0], b_bcs[0], not ln_triv[0], not ln_triv[0])
                pt = ps_tr.tile([HID, P], F32, space="PSUM", tag="tr")
                nc.tensor.transpose(out=pt[:, :nt], in_=h1[:nt, :HID], identity=ident[:nt, :nt])
                h1T = io.tile([HID, P], F32, tag="h1T")
                nc.vector.tensor_copy(h1T[:, :nt], pt[:, :nt])
                return h1, h1T

            # ---------- stage A1: kv1 table for ALL nodes (replicated) ----------
            for i in range(NTILE_N):
                g0 = i * P
                nt = min(P, N - g0)
                _, h1T = h1_tile(g0, nt)
                pkv = ps_mm.tile([P, 2 * C], F32, space="PSUM", tag="mm")
                nc.tensor.matmul(out=pkv[:nt], lhsT=h1T[:, :nt], rhs=Wkv1_sb[:],
                                 start=True, stop=True)
                kvs = io.tile([P, 2 * C], F32, tag="kvs")
                nc.vector.tensor_tensor(out=kvs[:nt], in0=pkv[:nt], in1=bkv1_bc[:nt],
                                        op=mybir.AluOpType.add)
                nc.sync.dma_start(out=kv1_tab[g0:g0 + nt, :], in_=kvs[:nt])

            # ---------- stage A2: q1 / s1|res1 for own shard ----------
            # own rows are [core*NSHARD, (core+1)*NSHARD); base passed via xT offset trick:
            # we cannot branch per-core, so host supplies own-shard x columns separately?
            # Instead: q1/sr1 tabs are local-indexed; host packs own-shard xT into xT1 too.
            # We simply recompute h1 from global columns own_base+b*P. own_base differs per
            # core; we avoid per-core code by gathering via dst indices instead.
            # Trick: host passes own-shard node features as extra input xTown.
            for b in range(NBLK):
                l0 = b * P
                nt = min(P, NSHARD - l0)
                xt = io.tile([IN_C + 1, P], F32, tag="xt")
                nc.sync.dma_start(out=xt[:, :nt], in_=xTown[:, l0:l0 + nt])
                ph = ps_mm.tile([P, HID], F32, space="PSUM", tag="mm")
                nc.tensor.matmul(out=ph[:nt], lhsT=xt[:, :nt], rhs=Winp_sb[:],
                                 start=True, stop=True)
                h1 = io.tile([P, HID], F32, tag="h1")
                _ln_relu(nc, lnp, h1, ph[:nt, :HID], nt, HID, eps_c,
                         g_bcs[0], b_bcs[0], not ln_triv[0], not ln_triv[0])
                pt = ps_tr.tile([HID, P], F32, space="PSUM", tag="tr")
                nc.tensor.transpose(out=pt[:, :nt], in_=h1[:nt, :HID], identity=ident[:nt, :nt])
                h1T = io.tile([HID, P], F32, tag="h1T")
                nc.vector.tensor_copy(h1T[:, :nt], pt[:, :nt])
                pq = ps_mm.tile([P, C], F32, space="PSUM", tag="mm")
                nc.tensor.matmul(out=pq[:nt], lhsT=h1T[:, :nt], rhs=Wq1_sb[:],
                                 start=True, stop=True)
                qs = io.tile([P, C], F32, tag="qs")
                nc.vector.tensor_tensor(out=qs[:nt], in0=pq[:nt], in1=bq1_bc[:nt],
                                        op=mybir.AluOpType.add)
                nc.sync.dma_start(out=q1_tab[l0:l0 + nt, :], in_=qs[:nt])
                psr = ps_mm.tile([P, 2 * C], F32, space="PSUM", tag="mm")
                nc.tensor.matmul(out=psr[:nt], lhsT=h1T[:, :nt], rhs=Wsr1_sb[:],
                                 start=True, stop=True)
                srs = io.tile([P, 2 * C], F32, tag="kvs")
                nc.vector.tensor_tensor(out=srs[:nt], in0=psr[:nt], in1=bsr1_bc[:nt],
                                        op=mybir.AluOpType.add)
                nc.sync.dma_start(out=sr1_tab[l0:l0 + nt, :], in_=srs[:nt])

            def edge_phase(kv_tab, q_tab, sr_tab, g_i, b_i, h_out_cb):
                for b in range(NBLK):
                    nt = min(P, NSHARD - b * P)
                    acc = ps_acc.tile([P, C + HEADS], F32, space="PSUM", tag="acc")
                    for t in range(T):
                        col = b * T + t
                        kvg = ep.tile([P, 2 * C], F32, tag="kvg")
                        nc.gpsimd.indirect_dma_start(
                            out=kvg[:], out_offset=None, in_=kv_tab[:],
                            in_offset=bass.IndirectOffsetOnAxis(ap=src_sb[:, col:col + 1], axis=0))
                        qg = ep.tile([P, C], F32, tag="qg")
                        nc.gpsimd.indirect_dma_start(
                            out=qg[:], out_offset=None, in_=q_tab[:],
                            in_offset=bass.IndirectOffsetOnAxis(ap=dst_sb[:, col:col + 1], axis=0))
                        qk = ep.tile([P, C], F32, tag="qk")
                        nc.vector.tensor_tensor(out=qk[:], in0=qg[:], in1=kvg[:, 0:C],
                                                op=mybir.AluOpType.mult)
                        mv = ep.tile([P, C + HEADS], F32, tag="mv")
                        nc.vector.tensor_reduce(out=mv[:, C:C + HEADS],
                                                in_=qk[:].rearrange("p (h d) -> p h d", h=HEADS),
                                                axis=mybir.AxisListType.X, op=mybir.AluOpType.add)
                        nc.scalar.activation(out=mv[:, C:C + HEADS], in_=mv[:, C:C + HEADS],
                                             func=mybir.ActivationFunctionType.Exp, scale=0.125)
                        nc.vector.tensor_tensor(
                            out=mv[:, 0:C].rearrange("p (h d) -> p h d", h=HEADS),
                            in0=kvg[:, C:2 * C].rearrange("p (h d) -> p h d", h=HEADS),
                            in1=mv[:, C:C + HEADS].to_broadcast([P, HEADS, 64]),
                            op=mybir.AluOpType.mult)
                        A = ep.tile([P, P], F32, tag="A")
                        nc.vector.tensor_tensor(out=A[:], in0=slot_sb[:, col:col + 1].to_broadcast([P, P]),
                                                in1=iota_bc[:], op=mybir.AluOpType.is_equal)
                        nc.tensor.matmul(out=acc[:], lhsT=A[:], rhs=mv[:],
                                         start=(t == 0), stop=(t == T - 1))
                    # finalize block
                    rc = fp.tile([P, HEADS], F32, tag="rc")
                    nc.vector.tensor_scalar(out=rc[:], in0=acc[:, C:C + HEADS],
                                            scalar1=1e-16, scalar2=None,
                                            op0=mybir.AluOpType.add)
                    nc.vector.reciprocal(rc[:], rc[:])
                    agg = fp.tile([P, C], F32, tag="agg")
                    nc.vector.tensor_tensor(
                        out=agg[:].rearrange("p (h d) -> p h d", h=HEADS),
                        in0=acc[:, 0:C].rearrange("p (h d) -> p h d", h=HEADS),
                        in1=rc[:].to_broadcast([P, HEADS, 64]),
                        op=mybir.AluOpType.mult)
                    sr = fp.tile([P, 2 * C], F32, tag="sr")
                    nc.sync.dma_start(out=sr[:nt], in_=sr_tab[b * P:b * P + nt, :])
                    nc.vector.tensor_tensor(out=agg[:nt], in0=agg[:nt], in1=sr[:nt, 0:C],
                                            op=mybir.AluOpType.add)
                    nc.vector.tensor_tensor(out=agg[:nt], in0=agg[:nt], in1=sr[:nt, C:2 * C],
                                            op=mybir.AluOpType.add)
                    hh = fp.tile([P, C], F32, tag="hh")
                    _ln_relu(nc, lnp, hh, agg[:nt, :C], nt, C, eps_c,
                             g_bcs[g_i], b_bcs[b_i], not ln_triv[g_i], not ln_triv[b_i])
                    h_out_cb(b, nt, hh)

            # ---------- conv1 ----------
            def h2_store(b, nt, hh):
                nc.sync.dma_start(out=h2_own[b * P:b * P + nt, :], in_=hh[:nt])
            edge_phase(kv1_tab, q1_tab, sr1_tab, 1, 1, h2_store)

            # ---------- allgather ----------
            nc.gpsimd.collective_compute(
                "AllGather", mybir.AluOpType.bypass,
                ins=[h2_own[:]], outs=[h2_full[:]],
                replica_groups=[list(range(NCORE))])

            def h2T_chunks(src_dram, r0, nt):
                hr = io.tile([P, C], F32, tag="hr")
                nc.sync.dma_start(out=hr[:nt], in_=src_dram[r0:r0 + nt, :])
                outs = []
                for k in range(2):
                    ptr = ps_tr.tile([P, P], F32, space="PSUM", tag="tr")
                    nc.tensor.transpose(out=ptr[:, :nt], in_=hr[:nt, k * P:(k + 1) * P],
                                        identity=ident[:nt, :nt])
                    hT = io.tile([P, P], F32, tag=f"hT{k}")
                    nc.vector.tensor_copy(hT[:, :nt], ptr[:, :nt])
                    outs.append(hT)
                return outs

            # ---------- conv2 tables ----------
            for i in range(NTILE_N):
                g0 = i * P
                nt = min(P, N - g0)
                hTa, hTb = h2T_chunks(h2_full, g0, nt)
                pkv = ps_mm.tile([P, 2 * C], F32, space="PSUM", tag="mm")
                nc.tensor.matmul(out=pkv[:nt], lhsT=hTa[:, :nt], rhs=Wkv2_sb[:, 0:2 * C],
                                 start=True, stop=False)
                nc.tensor.matmul(out=pkv[:nt], lhsT=hTb[:, :nt], rhs=Wkv2_sb[:, 2 * C:4 * C],
                                 start=False, stop=True)
                kvs = io.tile([P, 2 * C], F32, tag="kvs")
                nc.vector.tensor_tensor(out=kvs[:nt], in0=pkv[:nt], in1=bkv2_bc[:nt],
                                        op=mybir.AluOpType.add)
                nc.sync.dma_start(out=kv2_tab[g0:g0 + nt, :], in_=kvs[:nt])

            for b in range(NBLK):
                l0 = b * P
                nt = min(P, NSHARD - l0)
                hTa, hTb = h2T_chunks(h2_own, l0, nt)
                pq = ps_mm.tile([P, C], F32, space="PSUM", tag="mm")
                nc.tensor.matmul(out=pq[:nt], lhsT=hTa[:, :nt], rhs=Wq2_sb[:, 0:C],
                                 start=True, stop=False)
                nc.tensor.matmul(out=pq[:nt], lhsT=hTb[:, :nt], rhs=Wq2_sb[:, C:2 * C],
                                 start=False, stop=True)
                qs = io.tile([P, C], F32, tag="qs")
                nc.vector.tensor_tensor(out=qs[:nt], in0=pq[:nt], in1=bq2_bc[:nt],
                                        op=mybir.AluOpType.add)
                nc.sync.dma_start(out=q2_tab[l0:l0 + nt, :], in_=qs[:nt])
                psr = ps_mm.tile([P, 2 * C], F32, space="PSUM", tag="mm")
                nc.tensor.matmul(out=psr[:nt], lhsT=hTa[:, :nt], rhs=Wsr2_sb[:, 0:2 * C],
                                 start=True, stop=False)
                nc.tensor.matmul(out=psr[:nt], lhsT=hTb[:, :nt], rhs=Wsr2_sb[:, 2 * C:4 * C],
                                 start=False, stop=True)
                srs = io.tile([P, 2 * C], F32, tag="kvs")
                nc.vector.tensor_tensor(out=srs[:nt], in0=psr[:nt], in1=bsr2_bc[:nt],
                                        op=mybir.AluOpType.add)
                nc.sync.dma_start(out=sr2_tab[l0:l0 + nt, :], in_=srs[:nt])

            # ---------- conv2 + output head ----------
            def out_head(b, nt, hh):
                # hh = h3 [nt, C]; transpose chunks
                outs = []
                for k in range(2):
                    ptr = ps_tr.tile([P, P], F32, space="PSUM", tag="tr")
                    nc.tensor.transpose(out=ptr[:, :nt], in_=hh[:nt, k * P:(k + 1) * P],
                                        identity=ident[:nt, :nt])
                    hT = io.tile([P, P], F32, tag=f"hT{k}")
                    nc.vector.tensor_copy(hT[:, :nt], ptr[:, :nt])
                    outs.append(hT)
                po1 = ps_mm.tile([P, HID], F32, space="PSUM", tag="mm")
                nc.tensor.matmul(out=po1[:nt], lhsT=outs[0][:, :nt], rhs=Wo1_sb[:, 0:HID],
                                 start=True, stop=False)
                nc.tensor.matmul(out=po1[:nt], lhsT=outs[1][:, :nt], rhs=Wo1_sb[:, HID:2 * HID],
                                 start=False, stop=True)
                o1 = fp.tile([P, HID], F32, tag="o1")
                nc.vector.tensor_tensor(out=o1[:nt], in0=po1[:nt], in1=bo1_bc[:nt],
                                        op=mybir.AluOpType.add)
                h4 = fp.tile([P, HID], F32, tag="h4")
                _ln_relu(nc, lnp, h4, o1[:nt, :HID], nt, HID, eps_c,
                         g_bcs[3], b_bcs[3], not ln_triv[3], not ln_triv[3])
                pt = ps_tr.tile([HID, P], F32, space="PSUM", tag="tr")
                nc.tensor.transpose(out=pt[:, :nt], in_=h4[:nt, :HID], identity=ident[:nt, :nt])
                h4T = io.tile([HID, P], F32, tag="h1T")
                nc.vector.tensor_copy(h4T[:, :nt], pt[:, :nt])
                po2 = ps_mm.tile([P, OUT_C], F32, space="PSUM", tag="mm")
                nc.tensor.matmul(out=po2[:nt], lhsT=h4T[:, :nt], rhs=Wo2_sb[:],
                                 start=True, stop=True)
                ob = fp.tile([P, OUT_C], F32, tag="ob")
                nc.vector.tensor_tensor(out=ob[:nt], in0=po2[:nt], in1=bo2_bc[:nt],
                                        op=mybir.AluOpType.add)
                nc.sync.dma_start(out=out_sh[b * P:b * P + nt, :], in_=ob[:nt])

            edge_phase(kv2_tab, q2_tab, sr2_tab, 2, 2, out_head)

    nc.finalize()
    return nc


def _prep(x, params, edge_index):
    src = np.asarray(edge_index[0], dtype=np.int64)
    dst = np.asarray(edge_index[1], dtype=np.int64)
    core = (dst // NSHARD).astype(np.int64)
    per_core = []
    Tmax = 1
    for c in range(NCORE):
        sel = np.nonzero(core == c)[0]
        s = src[sel].astype(np.int32)
        dl = (dst[sel] - c * NSHARD).astype(np.int32)
        blk = dl // P
        order = np.argsort(blk, kind='stable')
        s, dl, blk = s[order], dl[order], blk[order]
        counts = np.bincount(blk, minlength=NBLK)
        Tmax = max(Tmax, int(np.ceil(counts.max() / P)))
        per_core.append((s, dl, blk, counts))
    T = Tmax
    NT_E = NBLK * T
    maps = []
    for c in range(NCORE):
        s, dl, blk, counts = per_core[c]
        srca = np.zeros(NT_E * P, np.int32)
        dsta = np.zeros(NT_E * P, np.int32)
        slota = np.full(NT_E * P, 999.0, np.float32)
        starts = np.concatenate([[0], np.cumsum(counts)[:-1]])
        rank = np.arange(len(s)) - starts[blk]
        pos = blk * T * P + rank
        srca[pos] = s
        dsta[pos] = dl
        slota[pos] = (dl % P).astype(np.float32)
        maps.append({
            "srci": np.ascontiguousarray(srca.reshape(NT_E, P).T),
            "dsti": np.ascontiguousarray(dsta.reshape(NT_E, P).T),
            "slotf": np.ascontiguousarray(slota.reshape(NT_E, P).T),
        })
    return T, maps


def _split256(W):
    return np.hstack([W[:P], W[P:]])


def kernel(x, params, edge_index):
    x = np.asarray(x, np.float32)
    T, edge_maps = _prep(x, params, edge_index)

    p = {k: {kk: np.asarray(vv, np.float32) for kk, vv in v.items()}
         for k, v in params.items()}
    lng = np.zeros((4, C), np.float32)
    lnb = np.zeros((4, C), np.float32)
    ln_names = ["ln_in", "ln1", "ln2", "ln_out"]
    ln_triv = []
    for r, nm in enumerate(ln_names):
        g, b_ = p[nm]["g"], p[nm]["b"]
        lng[r, :len(g)] = g
        lnb[r, :len(b_)] = b_
        ln_triv.append(bool(np.all(g == 1.0) and np.all(b_ == 0.0)))
    ln_triv = tuple(ln_triv)

    key = (x.shape, T, ln_triv)
    if key not in _CACHE:
        _CACHE[key] = _build(T, ln_triv)
    nc = _CACHE[key]

    xT1 = np.vstack([x.T, np.ones((1, N), np.float32)])
    common = {
        "xT1": np.ascontiguousarray(xT1),
        "Winp": np.vstack([p["inp"]["W"], p["inp"]["b"][None, :]]),
        "Wkv1": np.hstack([p["c1_k"]["W"], p["c1_v"]["W"]]),
        "bkv1": np.concatenate([p["c1_k"]["b"], p["c1_v"]["b"]])[None, :],
        "Wq1": p["c1_q"]["W"], "bq1": p["c1_q"]["b"][None, :],
        "Wsr1": np.hstack([p["c1_s"]["W"], p["res1"]["W"]]),
        "bsr1": np.concatenate([p["c1_s"]["b"], p["res1"]["b"]])[None, :],
        "Wkv2": _split256(np.hstack([p["c2_k"]["W"], p["c2_v"]["W"]])),
        "bkv2": np.concatenate([p["c2_k"]["b"], p["c2_v"]["b"]])[None, :],
        "Wq2": _split256(p["c2_q"]["W"]), "bq2": p["c2_q"]["b"][None, :],
        "Wsr2": _split256(np.hstack([p["c2_s"]["W"], p["res2"]["W"]])),
        "bsr2": np.concatenate([p["c2_s"]["b"], p["res2"]["b"]])[None, :],
        "Wo1": _split256(p["out1"]["W"]), "bo1": p["out1"]["b"][None, :],
        "Wo2": p["out2"]["W"], "bo2": p["out2"]["b"][None, :],
        "lng": lng, "lnb": lnb,
        "iota_d": np.arange(P, dtype=np.float32)[None, :],
    }
    common = {k: np.ascontiguousarray(v, np.float32) for k, v in common.items()}
    in_maps = []
    for c in range(NCORE):
        m = dict(common)
        m.update(edge_maps[c])
        # own-shard x columns (with ones row)
        m["xTown"] = np.ascontiguousarray(xT1[:, c * NSHARD:(c + 1) * NSHARD])
        in_maps.append(m)

    import time as _time
    _t0 = _time.time()
    r = run_bass_kernel_spmd(nc, in_maps, list(range(NCORE)))
    kernel._last_exec_s = _time.time() - _t0
    kernel._last_results = r
    out = np.concatenate([r.results[c]["out_sh"][:NSHARD] for c in range(NCORE)], axis=0)
    return out
